# revision 30
# baseline (speedup 1.0000x reference)
"""BoxConv2d Trainium2 kernel (band-sparse v2).

out[b, c*FN+f] = Wx[c,f] @ x[b,c] @ Wy[c,f]^T with clamped-ramp band matrices
(see kernel_v1_backup.py for the derivation).  This version exploits the BAND
structure of Wx/Wy: for a box of height h, the 256x256 band matrix has one
contiguous run of nonzero columns per 128-row chunk, so

  stage 1 (x side, psum V[j, f*256+io], contraction p in 2 chunks):
     per (jh, f-pair-bank): 2 matmuls  T(pc0, io in [0,W0), start=True) and
     BC(pc1, [L1,256), start=False), with 2-block column APs (the f-pair
     packed side by side; W0+W1 >= 256 guarantees coverage)
  stage 2 (y side, psum out[io, jo], contraction j in 2 chunks):
     per (f, ih): 2 matmuls  T(jc0, [0,G0), start=True)  BC(jc1, [M1,256))

This relies on the HW-verified whole-bank has_written clear: the start=True
matmul clears the bank's accumulate bits, so the start=False matmul adds in
the overlap region and overwrites (bits clear) in its exclusive region.
Merging keeps the column count minimal AND halves the LDWEIGHTS stream
(every matmul self-loads its stationary; LDWEIGHTS was the critical chain).

Stage 2 of (b,c) t is interleaved into the window of stage 1 of t+2 (vt
copies land in between) to keep the PE array fed.

The matmul column ranges are baked into the (single, SPMD) program, so ranges
are unified as max-over-cores; channels are assigned to core slots sorted by
band width, and filters permuted per channel, to keep the unified ranges tight.
Weights ship band-packed (only the nonzero column runs), ~2.6MB/core vs 4MB.
"""

import os
import numpy as np

B, C, FN, H, W = 4, 32, 4, 256, 256
N_CORES = 8
C_PER_CORE = C // N_CORES  # 4 channel slots per core

_PROGRAM_CACHE = {}

WARM_MMS = int(os.environ.get("BOXC_WARM", "72"))
LAG = int(os.environ.get("BOXC_LAG", "2"))  # stage2 interleave lag in (b,c) units


def _band(mn, mx, dim):
    """Overlap weights W[i, p] of clipped window [i+mn, i+mx+1) with cell
    [p, p+1), fp64."""
    i = np.arange(dim, dtype=np.float64)[:, None]
    p = np.arange(dim, dtype=np.float64)[None, :]
    lo = i + float(mn)
    hi = i + float(mx) + 1.0
    return np.clip(p + 1.0 - lo, 0.0, 1.0) - np.clip(p + 1.0 - hi, 0.0, 1.0)


def _pad8up(v):
    return int(min(256, (int(v) + 7) & ~7))


def _pad8dn(v):
    return int(max(0, int(v) & ~7))


def _support_cols(block):
    """[lo, hi) column support of a [128, 256] block (hi=0 if empty)."""
    nz = np.flatnonzero(np.abs(block).max(axis=0) > 0)
    if len(nz) == 0:
        return 0, 0
    return int(nz[0]), int(nz[-1]) + 1


def _plan(x_min, x_max, y_min, y_max):
    """Channel/filter assignment + unified column ranges + packed weights."""
    # Per (channel, filter) band matrices, transposed ([p, io] layout).
    WxT = np.empty((C, FN, 256, 256), np.float16)
    WyT = np.empty((C, FN, 256, 256), np.float16)
    h0 = np.zeros((C, FN), np.int32)   # x side: pc0 support is [0, h0)
    l1 = np.zeros((C, FN), np.int32)   # x side: pc1 support is [l1, 256)
    g0 = np.zeros((C, FN), np.int32)   # y side: jc0 support [0, g0)
    m1 = np.zeros((C, FN), np.int32)   # y side: jc1 support [m1, 256)
    for c in range(C):
        for f in range(FN):
            wx = _band(x_min[c, f], x_max[c, f], H).T
            wy = _band(y_min[c, f], y_max[c, f], W).T
            WxT[c, f] = wx.astype(np.float16)
            WyT[c, f] = wy.astype(np.float16)
            _, h = _support_cols(wx[0:128])
            lo, _ = _support_cols(wx[128:256])
            h0[c, f], l1[c, f] = max(h, 8), min(lo, 248)
            _, h = _support_cols(wy[0:128])
            lo, _ = _support_cols(wy[128:256])
            g0[c, f], m1[c, f] = max(h, 8), min(lo, 248)

    # Channel -> slot assignment and per-channel filter permutation, chosen
    # to minimize the unified (max-over-slot-members) matmul column counts.
    xext = h0 + (256 - l1)
    yext = g0 + (256 - m1)
    key = (xext + yext).sum(axis=1)
    order = np.argsort(-key, kind="stable")
    slot_of = np.zeros(C, np.int32)       # channel -> slot
    for cl in range(C_PER_CORE):
        for k in range(N_CORES):
            slot_of[order[8 * cl + k]] = cl
    perm = np.argsort(-xext, axis=1, kind="stable")  # [C, FN] start point

    from itertools import permutations
    ALLP = [np.array(p) for p in permutations(range(FN))]

    def slot_cost(members, perms):
        # members: channel ids in one slot; perms: their filter perms.
        # Stage-1 columns cost full PE time; stage-2 is near-LDWEIGHTS-bound,
        # so its column slack is weighted half.
        cost = 0.0
        for bank in range(2):
            hs = max(h0[ch, pm[2 * bank + i]] for ch, pm in zip(members, perms)
                     for i in (0, 1))
            ls = min(l1[ch, pm[2 * bank + i]] for ch, pm in zip(members, perms)
                     for i in (0, 1))
            cost += 4 * (256 + _pad8up(hs) - _pad8dn(ls))
        for fp in range(FN):
            gs = max(g0[ch, pm[fp]] for ch, pm in zip(members, perms))
            ms = min(m1[ch, pm[fp]] for ch, pm in zip(members, perms))
            cost += 2.0 * (256 + _pad8up(gs) - _pad8dn(ms))
        return cost

    def total_cost():
        tot = 0
        for cl in range(C_PER_CORE):
            mem = [ch for ch in range(C) if slot_of[ch] == cl]
            tot += slot_cost(mem, [perm[ch] for ch in mem])
        return tot

    def perm_sweep():
        for ch in range(C):
            cl = slot_of[ch]
            mem = [c2 for c2 in range(C) if slot_of[c2] == cl]
            best, bestp = None, None
            for p in ALLP:
                trial = [p if c2 == ch else perm[c2] for c2 in mem]
                cost = slot_cost(mem, trial)
                if best is None or cost < best:
                    best, bestp = cost, p
            perm[ch] = bestp

    # coordinate descent + restarted annealing on channel swaps
    rng = np.random.default_rng(0)
    perm_sweep()
    best_state = (total_cost(), slot_of.copy(), perm.copy())
    cur = best_state[0]
    for it in range(4000):
        a = int(rng.integers(C))
        b2 = int(rng.integers(C))
        if slot_of[a] == slot_of[b2]:
            continue
        slot_of[a], slot_of[b2] = slot_of[b2], slot_of[a]
        if it % 500 == 499:
            perm_sweep()
        cost = total_cost()
        temp = 60.0 * (1.0 - it / 4000.0) + 1.0
        if cost < cur or rng.random() < np.exp((cur - cost) / temp):
            cur = cost
            if cost < best_state[0]:
                best_state = (cost, slot_of.copy(), perm.copy())
        else:
            slot_of[a], slot_of[b2] = slot_of[b2], slot_of[a]
    slot_of[:] = best_state[1]
    perm[:] = best_state[2]
    perm_sweep()

    # slot members -> core order (any order works; keep sorted)
    assign = [[0] * C_PER_CORE for _ in range(N_CORES)]
    for cl in range(C_PER_CORE):
        mem = [ch for ch in range(C) if slot_of[ch] == cl]
        for k in range(N_CORES):
            assign[k][cl] = int(mem[k])

    # Unified (max-over-cores) ranges per (slot, position).
    W0u = np.zeros((C_PER_CORE, 2), np.int32)
    L1u = np.zeros((C_PER_CORE, 2), np.int32)
    G0u = np.zeros((C_PER_CORE, FN), np.int32)
    M1u = np.zeros((C_PER_CORE, FN), np.int32)
    for cl in range(C_PER_CORE):
        chans = [assign[k][cl] for k in range(N_CORES)]
        for bank in range(2):
            hs, ls = [], []
            for ch in chans:
                for fp in (2 * bank, 2 * bank + 1):
                    f = perm[ch][fp]
                    hs.append(h0[ch, f])
                    ls.append(l1[ch, f])
            W0u[cl, bank] = _pad8up(max(hs))
            L1u[cl, bank] = _pad8dn(min(ls))
        for fp in range(FN):
            gs, ms = [], []
            for ch in chans:
                f = perm[ch][fp]
                gs.append(g0[ch, f])
                ms.append(m1[ch, f])
            G0u[cl, fp] = _pad8up(max(gs))
            M1u[cl, fp] = _pad8dn(min(ms))

    # Weight layouts.  Per slot cl the x-weight columns are
    #   [bank0 A | bank1 A | bank0 B | bank1 B]  (A = pc0 pair 2*W0u cols,
    #   B = pc1 pair 2*W1u cols, W1u = 256 - L1u), A-blocks first so the
    #   startup load covers stage-1 pc0 of slot 0 quickly.
    xoffA = np.zeros((C_PER_CORE, 2), np.int64)
    xoffB = np.zeros((C_PER_CORE, 2), np.int64)
    xcl0 = np.zeros(C_PER_CORE + 1, np.int64)  # slot col start
    pos = 0
    for cl in range(C_PER_CORE):
        xcl0[cl] = pos
        for bank in range(2):
            xoffA[cl, bank] = pos
            pos += 2 * int(W0u[cl, bank])
        for bank in range(2):
            xoffB[cl, bank] = pos
            pos += 2 * (256 - int(L1u[cl, bank]))
    xcl0[C_PER_CORE] = pos
    XW = int(pos)

    yoff = np.zeros((C_PER_CORE, FN), np.int64)  # jc0 block start per (cl, f)
    ycl0 = np.zeros(C_PER_CORE + 1, np.int64)
    pos = 0
    for cl in range(C_PER_CORE):
        ycl0[cl] = pos
        for fp in range(FN):
            yoff[cl, fp] = pos
            pos += int(G0u[cl, fp]) + (256 - int(M1u[cl, fp]))
    ycl0[C_PER_CORE] = pos
    YW = int(pos)

    return dict(WxT=WxT, WyT=WyT, perm=perm, assign=assign,
                W0u=W0u, L1u=L1u, G0u=G0u, M1u=M1u,
                xoffA=xoffA, xoffB=xoffB, xcl0=xcl0, XW=XW,
                yoff=yoff, ycl0=ycl0, YW=YW)


def _range_key(plan):
    return (tuple(plan["W0u"].ravel()), tuple(plan["L1u"].ravel()),
            tuple(plan["G0u"].ravel()), tuple(plan["M1u"].ravel()),
            WARM_MMS, LAG)


def _build_program(plan):
    import concourse.bass as bass
    import concourse.tile as tile
    from concourse import bacc, mybir

    W0u, L1u = plan["W0u"], plan["L1u"]
    G0u, M1u = plan["G0u"], plan["M1u"]
    xoffA, xoffB = plan["xoffA"], plan["xoffB"]
    yoff = plan["yoff"]
    xcl0, ycl0 = plan["xcl0"], plan["ycl0"]
    XW, YW = plan["XW"], plan["YW"]

    nc = bacc.Bacc("TRN2", target_bir_lowering=False, debug=False)
    f16 = mybir.dt.float16
    f32 = mybir.dt.float32

    # xc[cl][p, b*512 + pc*256 + j] = x[b, ch(cl), pc*128+p, j]
    xcd = nc.dram_tensor("xc", [C_PER_CORE, 128, 2048], f16,
                         kind="ExternalInput").ap()
    wxd = nc.dram_tensor("wx", [128, XW], f16, kind="ExternalInput").ap()
    wyd = nc.dram_tensor("wy", [128, YW], f16, kind="ExternalInput").ap()
    # out[b, cl, p, fpos*512 + ih*256 + jo] = out[b, ch, ih*128+p, jo]
    out = nc.dram_tensor("out", [B, C_PER_CORE, 128, 2048], f16,
                         kind="ExternalOutput").ap()

    NT = B * C_PER_CORE  # 16 (b,c) units; t = cl*4 + b

    with tile.TileContext(nc, pool_alloc_mode="queue") as tc:
        with (
            tc.tile_pool(name="xc", bufs=2) as xc_pool,
            tc.tile_pool(name="wx", bufs=2) as wx_pool,
            tc.tile_pool(name="wy", bufs=2) as wy_pool,
            tc.tile_pool(name="vt", bufs=6) as vt_pool,
            tc.tile_pool(name="osb", bufs=3) as o_pool,
            tc.tile_pool(name="warm", bufs=1) as warm_pool,
            tc.tile_pool(name="psv", bufs=2, space=bass.MemorySpace.PSUM) as psv_pool,
            tc.tile_pool(name="pso", bufs=2, space=bass.MemorySpace.PSUM) as pso_pool,
        ):
            # ---- warm stationary: vector memset (gpsimd memset is slow
            # and would block the bulk-load queue ~3us) ------------------
            warm_sb = warm_pool.tile([128, 128], f16, tag="warm", name="warm")
            nc.vector.memset(warm_sb[:], 0.0)

            # ---- loads ----------------------------------------------------
            # First chunks on the two HWDGE queues (sync + scalar) for low
            # first-byte latency; the bulk on gpsimd (SWDGE).
            xc_t = [None] * C_PER_CORE
            wx_t = [None] * C_PER_CORE
            wy_t = [None] * C_PER_CORE
            for cl in range(C_PER_CORE):
                xc_t[cl] = xc_pool.tile([128, 2048], f16, tag="xc", name="xc")
                wx_t[cl] = wx_pool.tile([128, int(xcl0[cl + 1] - xcl0[cl])],
                                        f16, tag="wx", name="wx")
                wy_t[cl] = wy_pool.tile([128, int(ycl0[cl + 1] - ycl0[cl])],
                                        f16, tag="wy", name="wy")
            aw00 = int(xoffA[0, 1] - xcl0[0])  # bank0 A-block of slot 0
            aw0 = int(xoffB[0, 0] - xcl0[0])   # all A-blocks of slot 0
            nc.sync.dma_start(wx_t[0][:, :aw00], wxd[:, :aw00])
            nc.scalar.dma_start(xc_t[0][:, :256], xcd[0][:, :256])
            nc.sync.dma_start(wx_t[0][:, aw00:aw0], wxd[:, aw00:aw0])
            nc.scalar.dma_start(xc_t[0][:, 256:512], xcd[0][:, 256:512])
            bmid = int(xoffB[0, 1] - xcl0[0])
            nc.sync.dma_start(wx_t[0][:, aw0:bmid], wxd[:, aw0:bmid])
            nc.gpsimd.dma_start(wx_t[0][:, bmid:], wxd[:, bmid:int(xcl0[1])])
            nc.scalar.dma_start(xc_t[0][:, 512:768], xcd[0][:, 512:768])
            nc.scalar.dma_start(xc_t[0][:, 768:1024], xcd[0][:, 768:1024])
            nc.scalar.dma_start(xc_t[0][:, 1024:], xcd[0][:, 1024:])
            nc.gpsimd.dma_start(wy_t[0][:], wyd[:, :int(ycl0[1])])
            for cl in range(1, C_PER_CORE):
                nc.gpsimd.dma_start(wx_t[cl][:],
                                    wxd[:, int(xcl0[cl]):int(xcl0[cl + 1])])
                nc.gpsimd.dma_start(xc_t[cl][:], xcd[cl])
                nc.gpsimd.dma_start(wy_t[cl][:],
                                    wyd[:, int(ycl0[cl]):int(ycl0[cl + 1])])

            # ---- warmup ---------------------------------------------------
            warm_ps = pso_pool.tile([128, 1024], f32, tag="pso", name="pso")
            for _ in range(WARM_MMS):
                nc.tensor.matmul(warm_ps[:, :64], warm_sb[:], warm_sb[:, :64],
                                 start=True, stop=True)

            # ---- per-(b,c) emission --------------------------------------
            psv_tiles = {}   # t -> [psv_jh0, psv_jh1]
            vt_tiles = {}    # t -> [vt0, vt1]
            osb_tiles = {}
            pso_tiles = {}   # (t, fp) -> tile

            def s1_unit(t, jh, pc):
                """Stage-1 matmuls for one (jh, pc)."""
                cl, b = divmod(t, 4)
                if jh == 0 and pc == 0:
                    psv_tiles[t] = [
                        psv_pool.tile([128, 1024], f32, tag="psv", name="psv")
                        for _ in range(2)]
                psv = psv_tiles[t][jh]
                xt = xc_t[cl][:, b * 512 + pc * 256 + jh * 128:
                              b * 512 + pc * 256 + jh * 128 + 128]
                for bank in range(2):
                    W0 = int(W0u[cl, bank])
                    L1 = int(L1u[cl, bank])
                    W1 = 256 - L1
                    pblk = psv[:, bank * 512:(bank + 1) * 512].rearrange(
                        "p (g c) -> p g c", g=2)
                    if pc == 0:
                        rhs = wx_t[cl][:, int(xoffA[cl, bank] - xcl0[cl]):
                                       int(xoffA[cl, bank] - xcl0[cl]) + 2 * W0]
                        nc.tensor.matmul(pblk[:, :, 0:W0], xt, rhs,
                                         start=True, stop=False)
                    else:
                        boff = int(xoffB[cl, bank] - xcl0[cl])
                        wblk = wx_t[cl][:, boff:boff + 2 * W1].rearrange(
                            "p (g w) -> p g w", g=2)
                        nc.tensor.matmul(pblk[:, :, L1:256], xt,
                                         wblk[:, :, 0:W1],
                                         start=False, stop=True)

            def s1_copy(t, jh):
                if t not in vt_tiles:
                    vt_tiles[t] = [
                        vt_pool.tile([128, 1024], f16, tag="vt", name="vt")
                        for _ in range(2)]
                if jh == 1 and (t < 2 or t >= NT - 1):
                    # pipeline fill/drain: overlap the two copies across
                    # engines so stage 2 can start sooner
                    nc.scalar.copy(vt_tiles[t][jh][:], psv_tiles[t][jh][:])
                else:
                    nc.vector.tensor_copy(vt_tiles[t][jh][:],
                                          psv_tiles[t][jh][:])

            def s2_unit(t, fp, ih):
                """Stage-2 matmuls for one (f-position, ih)."""
                cl, b = divmod(t, 4)
                pair = fp // 2
                if fp % 2 == 0 and ih == 0:
                    pso_tiles[(t, pair)] = pso_pool.tile([128, 1024], f32,
                                                         tag="pso", name="pso")
                pso = pso_tiles[(t, pair)]
                G0 = int(G0u[cl, fp])
                M1 = int(M1u[cl, fp])
                yo0 = int(yoff[cl, fp] - ycl0[cl])
                yo1 = yo0 + G0
                vt0, vt1 = vt_tiles[t]
                st0 = vt0[:, fp * 256 + ih * 128: fp * 256 + ih * 128 + 128]
                st1 = vt1[:, fp * 256 + ih * 128: fp * 256 + ih * 128 + 128]
                o = (fp % 2) * 512 + ih * 256
                nc.tensor.matmul(pso[:, o:o + G0], st0,
                                 wy_t[cl][:, yo0:yo0 + G0],
                                 start=True, stop=False)
                nc.tensor.matmul(pso[:, o + M1:o + 256], st1,
                                 wy_t[cl][:, yo1:yo1 + 256 - M1],
                                 start=False, stop=True)

            def s2_copy(t, fp, eng_idx, whole_pair):
                cl, b = divmod(t, 4)
                if t not in osb_tiles:
                    osb_tiles[t] = o_pool.tile([128, 2048], f16, tag="o",
                                               name="osb")
                pair = fp // 2
                eng = nc.vector.tensor_copy if eng_idx == 0 else nc.scalar.copy
                if whole_pair:
                    dst = osb_tiles[t][:, pair * 1024:(pair + 1) * 1024]
                    eng(dst[:], pso_tiles[(t, pair)][:])
                else:
                    dst = osb_tiles[t][:, fp * 512:(fp + 1) * 512]
                    src = pso_tiles[(t, pair)][:, (fp % 2) * 512:
                                               (fp % 2) * 512 + 512]
                    eng(dst[:], src[:])

            def store(t, split):
                cl, b = divmod(t, 4)
                osb = osb_tiles[t]
                if split == 1:
                    nc.sync.dma_start(out[b, cl], osb[:])
                else:
                    stp = 2048 // split
                    for q in range(split):
                        nc.sync.dma_start(out[b, cl][:, q * stp:(q + 1) * stp],
                                          osb[:, q * stp:(q + 1) * stp])

            def emit_s1(t):
                """Full stage-1 of t as a unit list (callables)."""
                units = []
                for jh in range(2):
                    units.append(lambda t=t, jh=jh: s1_unit(t, jh, 0))
                    def u(t=t, jh=jh):
                        s1_unit(t, jh, 1)
                        s1_copy(t, jh)
                    units.append(u)
                return units

            def emit_s2(t, last=False):
                units = []
                for fp in range(FN):
                    units.append(lambda t=t, fp=fp: s2_unit(t, fp, 0))
                    def u(t=t, fp=fp, last=last):
                        s2_unit(t, fp, 1)
                        cl, b = divmod(t, 4)
                        if last and fp == 3:
                            # 2-piece final copy/store: big piece early on
                            # sync, tiny last piece on the idle scalar queue
                            # so only its small (store + receipt) chain tails
                            pso = pso_tiles[(t, 1)]
                            osb = osb_tiles[t]
                            nc.vector.tensor_copy(osb[:, 1536:1920][:],
                                                  pso[:, 512:896][:])
                            nc.sync.dma_start(out[b, cl][:, 1536:1920],
                                              osb[:, 1536:1920])
                            nc.scalar.copy(osb[:, 1920:2048][:],
                                           pso[:, 896:1024][:])
                            nc.scalar.dma_start(out[b, cl][:, 1920:2048],
                                                osb[:, 1920:2048])
                        elif last:
                            s2_copy(t, fp, fp % 2, False)
                            eng = nc.scalar if fp == 1 else nc.sync
                            eng.dma_start(
                                out[b, cl][:, fp * 512:(fp + 1) * 512],
                                osb_tiles[t][:, fp * 512:(fp + 1) * 512])
                        elif fp % 2 == 1:
                            # fused pair copies both on scalar (vector owns
                            # the latency-critical psv copies)
                            s2_copy(t, fp, 1, True)
                            if fp == 3:
                                store(t, 2 if t == NT - 2 else 1)
                    units.append(u)
                return units

            # Pacing: S2(t) units become eligible in window t+1 (vt(t) copies
            # land early there); each window drains the global S2 queue at a
            # rate that finishes by the end, so the post-S1 drain is short.
            s2q = []        # list of (t2, unit)
            appended = -1
            total_units = NT * 2 * FN
            done_units = 0
            for t in range(NT + LAG):
                s1u = emit_s1(t) if t < NT else []
                if t - 1 >= 0 and appended < t - 1 and t - 1 < NT:
                    s2q.extend(
                        emit_s2(t - 1, last=(t - 1 == NT - 1)))
                    appended = t - 1
                # strict lag-2 mid-stream; catch up in the last windows so
                # the post-S1 drain is one short window
                if t < 2:
                    want = 0
                elif t < NT:
                    # gentle catch-up: one extra unit over the last windows
                    # drains most of S2 before S1 ends without stalling on
                    # vt copies that land mid-window
                    want = 2 * FN + (1 if t >= NT - 8 else 0)
                else:
                    want = 99
                take_units = s2q[:want] if want < len(s2q) else list(s2q)
                del s2q[:len(take_units)]
                done_units += len(take_units)
                if not s1u:
                    for u in take_units:
                        u()
                elif not take_units:
                    for u in s1u:
                        u()
                else:
                    n1, n2 = len(s1u), len(take_units)
                    k1 = 0
                    for k2 in range(n2):
                        take_units[k2]()
                        take = ((k2 + 1) * n1) // n2 - k1
                        for _ in range(take):
                            s1u[k1]()
                            k1 += 1
                    while k1 < n1:
                        s1u[k1]()
                        k1 += 1
            for u in s2q:
                u()

    nc.compile()
    return nc


def _get_program(plan):
    key = _range_key(plan)
    if key not in _PROGRAM_CACHE:
        _PROGRAM_CACHE[key] = _build_program(plan)
    return _PROGRAM_CACHE[key]


def _prepare_in_maps(x, plan):
    x16 = x.astype(np.float16)
    WxT, WyT = plan["WxT"], plan["WyT"]
    perm, assign = plan["perm"], plan["assign"]
    W0u, L1u, G0u, M1u = plan["W0u"], plan["L1u"], plan["G0u"], plan["M1u"]
    xoffA, xoffB, yoff = plan["xoffA"], plan["xoffB"], plan["yoff"]
    XW, YW = plan["XW"], plan["YW"]

    in_maps = []
    for k in range(N_CORES):
        xc = np.zeros((C_PER_CORE, 128, 2048), np.float16)
        wx = np.zeros((128, XW), np.float16)
        wy = np.zeros((128, YW), np.float16)
        for cl in range(C_PER_CORE):
            ch = assign[k][cl]
            # xc[cl][p, b*512 + pc*256 + j]
            xc[cl] = (x16[:, ch].reshape(B, 2, 128, 256)
                      .transpose(2, 0, 1, 3).reshape(128, 2048))
            for bank in range(2):
                W0 = int(W0u[cl, bank])
                L1 = int(L1u[cl, bank])
                W1 = 256 - L1
                for i, fp in enumerate((2 * bank, 2 * bank + 1)):
                    f = int(perm[ch][fp])
                    a = int(xoffA[cl, bank]) + i * W0
                    wx[:, a:a + W0] = WxT[ch, f][0:128, 0:W0]
                    bo = int(xoffB[cl, bank]) + i * W1
                    wx[:, bo:bo + W1] = WxT[ch, f][128:256, L1:256]
            for fp in range(FN):
                f = int(perm[ch][fp])
                G0 = int(G0u[cl, fp])
                M1 = int(M1u[cl, fp])
                a = int(yoff[cl, fp])
                wy[:, a:a + G0] = WyT[ch, f][0:128, 0:G0]
                wy[:, a + G0:a + G0 + 256 - M1] = WyT[ch, f][128:256, M1:256]
        in_maps.append({"xc": xc, "wx": wx, "wy": wy})
    return in_maps


def run(input, x_min, x_max, y_min, y_max, trace=False):
    from concourse.bass_utils import run_bass_kernel_spmd

    x = np.asarray(input, dtype=np.float32)
    plan = _plan(np.asarray(x_min, np.float64), np.asarray(x_max, np.float64),
                 np.asarray(y_min, np.float64), np.asarray(y_max, np.float64))
    nc = _get_program(plan)
    in_maps = _prepare_in_maps(x, plan)
    res = run_bass_kernel_spmd(nc, in_maps, list(range(N_CORES)), trace=trace)

    perm, assign = plan["perm"], plan["assign"]
    full = np.empty((B, C * FN, 256, 256), np.float32)
    for k in range(N_CORES):
        o = res.results[k]["out"].astype(np.float32)
        # o[b, cl, p, fp*512 + ih*256 + jo]
        o = o.reshape(B, C_PER_CORE, 128, FN, 2, 256)
        o = o.transpose(0, 1, 3, 4, 2, 5)  # [b, cl, fp, ih, p, jo]
        for cl in range(C_PER_CORE):
            ch = assign[k][cl]
            idx = ch * FN + perm[ch]  # output channel per f-position
            full[:, idx] = o[:, cl].reshape(B, FN, 256, 256)
    return full, res


def kernel(input, x_min, x_max, y_min, y_max):
    full, _ = run(input, x_min, x_max, y_min, y_max)
    return full
